# revision 9
# baseline (speedup 1.0000x reference)
"""ContraNorm kernel for 8 Trainium2 NeuronCores.

Math (reference):
    norm_x = x / max(||x||_row, eps)
    sim    = (norm_x @ norm_x.T) / tau          # [N, N], tau = 1
    sim[edge_index[0], edge_index[1]] = -inf
    attn   = softmax(sim, axis=1)
    out    = 1.1 * x - 0.1 * (attn @ x)

Sharding: row-parallel, flash-attention style.  Core k owns output rows
[k*1024, (k+1)*1024).  Each core receives the FULL x but row-rolled so that
its own 1024 rows sit at positions 0:1024 — that makes the program identical
on every core (pure SPMD, no partition-id); only the input data differs.

Because sim entries are cosine similarities in [-1, 1], softmax needs no
running-max: exp(sim) is in [e^-1, e].  The -inf edge mask becomes an exact
multiply by a {0, 1} mask applied to exp(sim).  The row-sum comes for free
from a ones-column appended to the V-matmul rhs.

Per-core device pipeline (c = key/source index, m = this core's 1024 rows):
  preamble: ssq per row -> sqrt -> 1/x;  norm_x bf16; PE-transpose into
            nxT [d, c] layout; x (+ones col) as V rhs in [c, d+1] layout.
  main, for each m-half (512 cols) and c-chunk (128 rows):
      psum_sim[c,m] = nxT_d0.T@nxT + nxT_d1.T@nxT     (2 bf16 matmuls)
      expT = exp(psum_sim)     (ScalarE, PSUM->SBUF bf16)
      expT *= maskT tile       (DVE, bf16 2x mode)
      psum_V[m, 0:257] += expT_chunk.T @ [x | 1]      (4 bf16 matmuls, accum)
  epilogue per 128-row m-chunk:
      S = psum_V[:, 256]; out = (1+s)*x_own - s * psum_V[:, 0:256]/S
"""

import numpy as np
import ml_dtypes

N = 8192          # rows of x
D = 256           # features
P = 128           # SBUF partitions
NT = N // P       # 64 c-chunks
R = N // 8        # 1024 rows per core
RT = R // P       # 8 m-chunks per core
HALF = 512        # m columns per pass
SCALE = 0.1
NCORES = 8
GB = 8            # c-chunks batched per DMA (1 MiB loads)

_prog_cache = {}


def _build_program(nreps=1, mask_split=True):
    import concourse.bacc as bacc
    import concourse.tile as tile
    from concourse import mybir
    from concourse.masks import make_identity
    from contextlib import ExitStack

    f32 = mybir.dt.float32
    bf16 = mybir.dt.bfloat16
    AX = mybir.AxisListType.X
    MUL = mybir.AluOpType.mult
    Exp = mybir.ActivationFunctionType.Exp
    Sqrt = mybir.ActivationFunctionType.Sqrt

    nc = bacc.Bacc("TRN2", target_bir_lowering=False, debug=False)

    xb_h = nc.dram_tensor("xb", [N, D], bf16, kind="ExternalInput")
    xo_h = nc.dram_tensor("xo", [R, D], f32, kind="ExternalInput")
    mk_h = nc.dram_tensor("maskT", [N, R], bf16, kind="ExternalInput")
    out_h = nc.dram_tensor("out", [R, D], f32, kind="ExternalOutput")

    xb = xb_h.ap().rearrange("(t p) d -> p t d", p=P)      # [128, 64, 256]
    xo_d = xo_h.ap().rearrange("(j p) d -> p j d", p=P)    # [128, 8, 256]
    mk = mk_h.ap().rearrange("(t p) m -> p t m", p=P)      # [128, 64, 1024]
    out_d = out_h.ap()

    with ExitStack() as ctx:
        tc = ctx.enter_context(tile.TileContext(nc))

        consts = ctx.enter_context(tc.tile_pool(name="consts", bufs=1))
        pre = ctx.enter_context(tc.tile_pool(name="pre", bufs=3))
        work = ctx.enter_context(tc.tile_pool(name="work", bufs=4))
        ps_t = ctx.enter_context(tc.tile_pool(name="ps_t", bufs=2, space="PSUM"))
        ps_s = ctx.enter_context(tc.tile_pool(name="ps_s", bufs=2, space="PSUM"))
        ps_v = ctx.enter_context(tc.tile_pool(name="ps_v", bufs=1, space="PSUM"))

        xa = consts.tile([P, NT, D + 1], bf16)    # V rhs: [x | 1] per c-chunk
        nxT = consts.tile([P, 2, N], bf16)        # norm_x transposed [d, c]
        xo = consts.tile([P, RT, D], f32)         # own rows, f32, for epilogue
        ident = consts.tile([P, P], bf16)
        ssq = consts.tile([P, NT], f32)
        inv = consts.tile([P, NT], f32)

        make_identity(nc, ident)
        nc.sync.dma_start(out=xo, in_=xo_d)
        nc.gpsimd.memset(xa[:, :, D : D + 1], 1.0)

        for _rep in range(nreps):
            _emit_body(nc, tile, mybir, pre, work, ps_t, ps_s, ps_v,
                       xa, nxT, xo, ident, ssq, inv, xb, mk, out_d,
                       mask_split)

    nc.compile()
    return nc


def _emit_body(nc, tile, mybir, pre, work, ps_t, ps_s, ps_v,
               xa, nxT, xo, ident, ssq, inv, xb, mk, out_d, mask_split):
    f32 = mybir.dt.float32
    bf16 = mybir.dt.bfloat16
    MUL = mybir.AluOpType.mult
    Exp = mybir.ActivationFunctionType.Exp
    Sqrt = mybir.ActivationFunctionType.Sqrt

    if True:
        # ---- preamble: row norms + transposed normalized x ----
        for g in range(NT // GB):
            sl = slice(g * GB, (g + 1) * GB)
            # straight into the V rhs layout (strided dst, no extra copy)
            nc.sync.dma_start(out=xa[:, sl, 0:D], in_=xb[:, sl, :])
            for j in range(GB):
                t = g * GB + j
                sq = pre.tile([P, D], bf16, tag="sq")
                nc.vector.scalar_tensor_tensor(
                    out=sq,
                    in0=xa[:, t, 0:D],
                    scalar=1.0,
                    in1=xa[:, t, 0:D],
                    op0=MUL,
                    op1=MUL,
                    accum_out=ssq[:, t : t + 1],
                )
            std = pre.tile([P, GB], f32, tag="std")
            nc.scalar.activation(std, ssq[:, sl], Sqrt)
            nc.vector.reciprocal(inv[:, sl], std)
            nx8 = pre.tile([P, GB, D], bf16, tag="nx8")
            for j in range(GB):
                t = g * GB + j
                nc.vector.tensor_scalar_mul(
                    nx8[:, j], xa[:, t, 0:D], inv[:, t : t + 1]
                )
            for h in range(2):
                for q in range(2):
                    tp4 = ps_t.tile([P, 4, P], bf16, tag="tp4")
                    for jj in range(4):
                        nc.tensor.transpose(
                            tp4[:, jj],
                            nx8[:, q * 4 + jj, h * P : (h + 1) * P],
                            ident,
                        )
                    c0 = (g * GB + q * 4) * P
                    nc.vector.tensor_copy(
                        out=nxT[:, h, c0 : c0 + 4 * P], in_=tp4
                    )

        # ---- main: two passes over this core's 1024 rows (512 each) ----
        for half in range(2):
            m0 = half * HALF
            pv = [
                ps_v.tile([P, D + 1], f32, tag=f"pv{i}", name=f"pv{i}")
                for i in range(4)
            ]
            for g in range(NT // GB):
                mk8 = work.tile([P, GB, HALF], bf16, tag="mk8")
                nc.sync.dma_start(
                    out=mk8, in_=mk[:, g * GB : (g + 1) * GB, m0 : m0 + HALF]
                )
                for j in range(GB):
                    t = g * GB + j
                    ps = ps_s.tile([P, HALF], f32, tag="ps")
                    nc.tensor.matmul(
                        ps,
                        nxT[:, 0, t * P : (t + 1) * P],
                        nxT[:, 0, m0 : m0 + HALF],
                        start=True,
                        stop=False,
                    )
                    nc.tensor.matmul(
                        ps,
                        nxT[:, 1, t * P : (t + 1) * P],
                        nxT[:, 1, m0 : m0 + HALF],
                        start=False,
                        stop=True,
                    )
                    et = work.tile([P, HALF], bf16, tag="et", bufs=6)
                    nc.scalar.activation(et, ps, Exp)
                    # optionally split the mask multiply across DVE and GpSimd
                    eng = (
                        nc.gpsimd
                        if (mask_split and j % 2 == 1)
                        else nc.vector
                    )
                    eng.tensor_mul(et, et, mk8[:, j])
                    for mi in range(4):
                        nc.tensor.matmul(
                            pv[mi],
                            et[:, mi * P : (mi + 1) * P],
                            xa[:, t, :],
                            start=(t == 0),
                            stop=(t == NT - 1),
                        )
            for mi in range(4):
                jj = half * 4 + mi
                sinv = work.tile([P, 1], f32, tag="sinv")
                nc.vector.reciprocal(sinv, pv[mi][:, D : D + 1])
                res = work.tile([P, D], f32, tag="res")
                nc.vector.tensor_scalar(
                    out=res,
                    in0=pv[mi][:, 0:D],
                    scalar1=sinv,
                    scalar2=-SCALE,
                    op0=MUL,
                    op1=MUL,
                )
                t1 = work.tile([P, D], f32, tag="t1")
                nc.vector.tensor_scalar_mul(t1, xo[:, jj], 1.0 + SCALE)
                nc.vector.tensor_add(res, res, t1)
                nc.sync.dma_start(
                    out=out_d[jj * P : (jj + 1) * P, :], in_=res
                )


def get_program(nreps=1, mask_split=True):
    key = (nreps, mask_split)
    if key not in _prog_cache:
        _prog_cache[key] = _build_program(nreps, mask_split)
    return _prog_cache[key]


def make_in_maps(x, edge_index):
    x = np.asarray(x, dtype=np.float32)
    ei = np.asarray(edge_index)
    r = ei[0].astype(np.int64)
    c = ei[1].astype(np.int64)
    in_maps = []
    for k in range(NCORES):
        lo = k * R
        xb = np.roll(x, -lo, axis=0).astype(ml_dtypes.bfloat16)
        xo = np.ascontiguousarray(x[lo : lo + R])
        sel = (r >= lo) & (r < lo + R)
        m_local = r[sel] - lo
        c_rolled = (c[sel] - lo) % N
        mask = np.ones((N, R), dtype=ml_dtypes.bfloat16)
        mask[c_rolled, m_local] = 0
        in_maps.append({"xb": xb, "xo": xo, "maskT": mask})
    return in_maps


def run(x, edge_index, trace=False):
    from concourse.bass_utils import run_bass_kernel_spmd

    nc = get_program()
    in_maps = make_in_maps(x, edge_index)
    br = run_bass_kernel_spmd(nc, in_maps, list(range(NCORES)), trace=trace)
    out = np.concatenate(
        [br.results[k]["out"] for k in range(NCORES)], axis=0
    ).astype(np.float32)
    return out, br


def kernel(x, edge_index):
    out, _ = run(x, edge_index, trace=False)
    return out
